# revision 37
# baseline (speedup 1.0000x reference)
"""Distributed Trainium2 kernel for nn_Attention_14697378086932.

Head-sharded (tensor-parallel) multi-head attention over 8 NeuronCores:
each core computes 2 of the 16 heads end-to-end.

Phase A (serial prefix, jointly DMA/PE-paced at ~420GB/s): QKV
projections for both batches, 512-token chunk pairs.  x^T arrives
host-rearranged so each chunk is one contiguous 16KB line per partition
(fewest DMA descriptors), split into two o-half tiles so the first
matmul waits on 1MB, not 2MB; 7 half-tile buffers keep the DMA engines
streaming continuously.  K's activation writes straight into the
zero-padded per-head KzA/KzB tiles, V is written bf16 and PE-transposed
(one bf16 transpose per 128-token chunk) into the per-key-chunk
[token, 1|pad|64ch] PV layout whose ones column drops the softmax
denominator onto PSUM partition 0.  rotate_half is a permutation
matmul; cores 1..7 get cos=1/sin=0 so rope degenerates to the
identity.  Pair p-1's rope runs BEFORE pair p's projections and its V
transposes after them, so every PSUM-bank WAR clears inside the
DMA-paced stream and the in-order PE queue never stalls.

Attention blocks (ScalarE exp-bound, ~16.4us per 1024-q block):
flash-style over 128-token key chunks; S^T = Kz Q^T (f32r), P^T =
exp(S^T) on ScalarE, O^T = [1|V]^T P^T (bf16) accumulating rows 64:128
of PSUM with den on row 0.  The S/exp/PV pipeline carries across block
boundaries (DEPTH=5); normalize = offset-blind fast approx reciprocal
read directly from PSUM row 0, gpsimd partition broadcast, and one
PSUM-direct multiply into the 512-token-granular OtT tiles.  Output
projections (O_loc @ Wo per 128-token chunk, stationary-shared halves
on the spP/spD banks, bf16 partials) inject into later blocks at age
>= 2; the final block is split into 512-q halves so only 4 tiles drain
at the end, round-robin over three PSUM banks with copies alternating
ScalarE/DVE.  Host sums the 8 partial outputs and adds bo.

All tiles are split per batch (and Kz/Vaug/Qt per 1024-token half) so
Tile's per-tile dependency tracking never chains a consumer onto an
unrelated producer.  All matmuls keep a full 128-row/column PE
footprint (the clock gate throttles half-array work to 1.2 GHz).
NOTE: the device alternates between two clock states (~1.2x apart), so
wall-clock exec times vary run to run; ~254-261us in the fast state.
"""
import sys

sys.path.insert(0, "/opt/trn_rl_repo")

import numpy as np
import ml_dtypes

import concourse.bass as bass
import concourse.mybir as mybir
from concourse import bacc
from concourse.bass import ts, ds
from concourse.tile import TileContext
from concourse.masks import make_identity
from concourse.bass_utils import run_bass_kernel_spmd

F32 = mybir.dt.float32
F32R = mybir.dt.float32r
BF16 = mybir.dt.bfloat16

P = 128          # partitions / local channels per core
HID = 1024       # hidden
NT = 4096        # total tokens (batch 2 x 2048)
NB = 2048        # tokens per batch
HD = 64          # head dim
N_CORES = 8

_NC_CACHE = None


def build_nc():
    nc = bacc.Bacc("TRN2")

    # xt: host-rearranged [p, chunk, o, n_tail]: each 512-token chunk is
    # one contiguous 16KB line per partition (fewest DMA descriptors)
    xt = nc.declare_dram_parameter("xt", [P, 8, 8, 512], F32R, isOutput=False)
    wq = nc.declare_dram_parameter("wq", [P, 8, P], F32R, isOutput=False)
    wk = nc.declare_dram_parameter("wk", [P, 8, P], F32R, isOutput=False)
    wv = nc.declare_dram_parameter("wv", [P, 8, P], F32R, isOutput=False)
    wo = nc.declare_dram_parameter("wo", [P, HID], BF16, isOutput=False)
    bia = nc.declare_dram_parameter("bias", [P, 3], F32, isOutput=False)
    cos = nc.declare_dram_parameter("cos", [HD, NT], BF16, isOutput=False)
    sin = nc.declare_dram_parameter("sin", [HD, NT], BF16, isOutput=False)
    rmat = nc.declare_dram_parameter("rmat", [P, P], F32R, isOutput=False)
    out = nc.declare_dram_parameter("out", [NT, HID], BF16, isOutput=True)

    with TileContext(nc) as tc:
        with tc.tile_pool(name="consts", bufs=1) as consts, \
             tc.tile_pool(name="big", bufs=1) as big, \
             tc.tile_pool(name="xtp", bufs=7) as xtp, \
             tc.tile_pool(name="ropet", bufs=1) as ropet, \
             tc.tile_pool(name="ptp", bufs=7) as ptp, \
             tc.tile_pool(name="osb", bufs=2) as osb, \
             tc.tile_pool(name="nrm", bufs=1) as nrm, \
             tc.tile_pool(name="spS", bufs=2, space="PSUM") as spS, \
             tc.tile_pool(name="spO", bufs=1, space="PSUM") as spO, \
             tc.tile_pool(name="spP", bufs=1, space="PSUM") as spP, \
             tc.tile_pool(name="spD", bufs=1, space="PSUM") as spD:

            # ---------------- constants (Activation HWDGE queue, parallel
            # with the xt stream on the SP queue)
            wqs = consts.tile([P, 8, P], F32R)
            wks = consts.tile([P, 8, P], F32R)
            wvs = consts.tile([P, 8, P], F32R)
            nc.scalar.dma_start(wqs, wq[:])
            nc.scalar.dma_start(wks, wk[:])
            nc.scalar.dma_start(wvs, wv[:])
            bias_t = consts.tile([P, 3], F32)
            nc.scalar.dma_start(bias_t, bia[:])
            rmat_t = consts.tile([P, P], F32R)
            nc.scalar.dma_start(rmat_t, rmat[:])
            trig_t = consts.tile([P, NT], BF16)
            cos_t = trig_t[0:HD]
            sin_t = trig_t[HD:P]
            nc.scalar.dma_start(cos_t, cos[:])
            nc.scalar.dma_start(sin_t, sin[:])
            wos = consts.tile([P, HID], BF16)
            nc.scalar.dma_start(wos, wo[:])
            ident = consts.tile([P, P], BF16)
            make_identity(nc, ident)

            # ---------------- per-batch state, split into 1024-token
            # half tiles so consumers wait only on the half they read
            # Q^T per (batch, nq-half) [128 ch, 1024 tok]
            Qt = [[big.tile([P, 1024], F32R, name=f"Qt{b}{n}")
                   for n in range(2)] for b in range(2)]
            # zero-padded per-head K^T: head A in rows 0..63, head B in
            # rows 64..127; the K activation writes its halves directly
            # into these (no intermediate Kt tile)
            KzA = [[big.tile([P, 1024], F32R, name=f"KzA{b}{n}")
                    for n in range(2)] for b in range(2)]
            KzB = [[big.tile([P, 1024], F32R, name=f"KzB{b}{n}")
                    for n in range(2)] for b in range(2)]
            for b in range(2):
                for n in range(2):
                    nc.vector.memset(KzA[b][n][HD:P, :].bitcast(F32), 0.0)
                    nc.vector.memset(KzB[b][n][0:HD, :].bitcast(F32), 0.0)
            # V^T staging in bf16, shared across batches (batch 1's
            # projection rewrites it only after batch 0's transposes read it)
            _vtb = big.tile([P, NB], BF16, name="Vtb")
            Vtb = [_vtb, _vtb]
            # V in [token, chunk, 64 V | 1 | 63 zeros] layout per
            # (head, batch, 8-chunk half)
            VaugA = [[big.tile([P, 8, P], BF16, name=f"VaugA{b}{n}")
                      for n in range(2)] for b in range(2)]
            VaugB = [[big.tile([P, 8, P], BF16, name=f"VaugB{b}{n}")
                      for n in range(2)] for b in range(2)]
            # col 0 = ones (so the softmax denominator lands on PSUM
            # partition 0, readable by the offset-blind custom reciprocal
            # with no staging copy); V channels in cols 64:128
            for b in range(2):
                for V in (*VaugA[b], *VaugB[b]):
                    nc.vector.memset(V, 0.0)
                    nc.vector.memset(V[:, :, 0:1], 1.0)
            # normalized attention out^T, one tile per 512 tokens so
            # output projections unblock at fine grain
            OtT = [big.tile([P, 512], BF16, name=f"Ot{k}") for k in range(8)]

            # ---------------- building blocks
            def load_chunk(c):
                # one 512-token chunk of x^T as two [128, 4 o-chunks, 512]
                # tiles so the first projection matmuls wait on 1MB, not 2MB
                ta = xtp.tile([P, 4, 512], F32R, tag="xt", name="xta")
                nc.sync.dma_start(ta, xt[:, c, 0:4])
                tb = xtp.tile([P, 4, 512], F32R, tag="xt", name="xtb")
                nc.sync.dma_start(tb, xt[:, c, 4:8])
                return (ta, tb)

            def qkv_pair(xtt0, xtt1, wt, acc, c0):
                # one projection for a pair of 512-token chunks into a
                # 2-bank accumulator; paired matmuls share their stationary
                # so every second one skips LDWEIGHTS
                for o in range(8):
                    nc.tensor.matmul(acc[:, 0:512], wt[:, o],
                                     xtt0[o // 4][:, o % 4],
                                     start=(o == 0), stop=(o == 7))
                    nc.tensor.matmul(acc[:, 512:1024], wt[:, o],
                                     xtt1[o // 4][:, o % 4],
                                     start=(o == 0), stop=(o == 7))

            def act_q(acc, b, l0, ln, dve=False):
                dst = Qt[b][l0 // 1024][:, ds(l0 % 1024, ln)]
                if dve:
                    nc.vector.tensor_scalar_add(dst, acc, bias_t[:, 0:1])
                else:
                    nc.scalar.activation(dst, acc,
                                         mybir.ActivationFunctionType.Identity,
                                         bias=bias_t[:, 0:1])

            def act_k(acc, b, l0, ln, dve=False):
                dA = KzA[b][l0 // 1024][0:HD, ds(l0 % 1024, ln)]
                dB = KzB[b][l0 // 1024][HD:P, ds(l0 % 1024, ln)]
                if dve:
                    nc.vector.tensor_scalar_add(dA, acc[0:HD],
                                                bias_t[0:HD, 1:2])
                    nc.vector.tensor_scalar_add(dB, acc[HD:P],
                                                bias_t[HD:P, 1:2])
                else:
                    nc.scalar.activation(dA, acc[0:HD],
                                         mybir.ActivationFunctionType.Identity,
                                         bias=bias_t[0:HD, 1:2])
                    nc.scalar.activation(dB, acc[HD:P],
                                         mybir.ActivationFunctionType.Identity,
                                         bias=bias_t[HD:P, 1:2])

            def act_v(acc, b, l0, ln, dve=False):
                if dve:
                    nc.vector.tensor_scalar_add(Vtb[b][:, ds(l0, ln)], acc,
                                                bias_t[:, 2:3])
                else:
                    nc.scalar.activation(Vtb[b][:, ds(l0, ln)], acc,
                                         mybir.ActivationFunctionType.Identity,
                                         bias=bias_t[:, 2:3])

            def rope_slice(tiles, b, l0, psr):
                # rope rows 0..63 of the 512-token slice at local offset l0
                t = tiles[b][l0 // 1024]
                lsl = ds(l0 % 1024, 512)
                gsl = ds(b * NB + l0, 512)
                nc.tensor.matmul(psr, rmat_t, t[:, lsl],
                                 start=True, stop=True)
                tmp = ropet.tile([HD, 512], F32, tag="tmp", name="tmp")
                nc.vector.tensor_tensor(tmp, psr[0:HD], sin_t[:, gsl],
                                        mybir.AluOpType.mult)
                nc.vector.tensor_tensor(t[0:HD, lsl], t[0:HD, lsl],
                                        cos_t[:, gsl], mybir.AluOpType.mult)
                nc.vector.tensor_tensor(t[0:HD, lsl], t[0:HD, lsl], tmp,
                                        mybir.AluOpType.add)

            def vtrans_chunk(b, kc, pst_bf, slot):
                # V [ch, tok] -> [tok, ch] per 128-token chunk: one bf16 PE
                # transpose (both heads at once) into a bf16 view of a
                # shared PSUM bank, then two copies into the Vaug layouts
                dst = pst_bf[:, ts(slot, P)]
                nc.tensor.transpose(dst, Vtb[b][:, ts(kc, P)], ident)
                nc.vector.tensor_copy(VaugA[b][kc // 8][:, kc % 8, HD:P],
                                      dst[:, 0:HD])
                nc.vector.tensor_copy(VaugB[b][kc // 8][:, kc % 8, HD:P],
                                      dst[:, HD:P])

            # ---------------- phase A: projections for both batches
            # (serial prefix, DMA-bound).  Software pipeline: pair p-1's
            # rope runs BEFORE pair p's projection matmuls and its V
            # transposes after them, so every PSUM-bank WAR clears during
            # the long DMA-paced projection stream and the in-order PE
            # queue never stalls on the DVE chains.
            def rope_pair(pr):
                b, l0p = pr // 2, (pr % 2) * 1024
                for u in range(2):
                    l0 = l0p + u * 512
                    psr = spP.tile([P, 512], F32, tag="oproj", name="psrQ")
                    rope_slice(Qt, b, l0, psr)
                    psr2 = spD.tile([P, 512], F32, tag="dummy", name="psrK")
                    rope_slice(KzA, b, l0, psr2)

            def vtrans_pair(pr):
                b, kc0 = pr // 2, (pr % 2) * 8
                for u, (pool, tag) in enumerate(((spP, "oproj"),
                                                 (spD, "dummy"))):
                    pst = pool.tile([P, 512], F32, tag=tag, name="pstV")
                    pst_bf = pst.bitcast(BF16)
                    for s in range(4):
                        vtrans_chunk(b, kc0 + u * 4 + s, pst_bf, s)

            xtts = [load_chunk(c) for c in range(2)]
            for pr in range(4):           # chunk pairs per batch: 2 x 2
                b = pr // 2
                c0 = 2 * pr
                if pr > 0:
                    rope_pair(pr - 1)
                x0, x1 = xtts[0], xtts[1]
                accQ = spS.tile([P, 1024], F32, tag="S", name="accQ")
                qkv_pair(x0, x1, wqs, accQ, c0)
                accK = spS.tile([P, 1024], F32, tag="S", name="accK")
                qkv_pair(x0, x1, wks, accK, c0)
                accV = spO.tile([P, 1024], F32, tag="O", name="accV")
                qkv_pair(x0, x1, wvs, accV, c0)
                xtts = xtts[2:]
                for c in (c0 + 2, c0 + 3):
                    if c < 8:
                        xtts.append(load_chunk(c))
                l0p = (pr % 2) * 1024
                act_q(accQ, b, l0p, 1024)
                act_k(accK, b, l0p, 1024)
                act_v(accV, b, l0p, 1024)
                if pr > 0:
                    vtrans_pair(pr - 1)
            rope_pair(3)
            vtrans_pair(3)

            side_work = {}

            # ---------------- attention + output projection
            def oproj_tile(t0):
                # output projection of one 128-token chunk (both heads);
                # the two halves use different psum banks so the second
                # matmul never queues behind the first half's PSUM read
                lhs = OtT[t0 // 512][:, ts((t0 % 512) // P, P)]
                ost = osb.tile([P, HID], BF16, tag="ost", name="ost")
                Pps = spP.tile([P, 512], F32, tag="oproj", name="opj")
                nc.tensor.matmul(Pps, lhs, wos[:, 0:512],
                                 start=True, stop=True)
                nc.any.tensor_copy(ost[:, 0:512], Pps)
                Pps2 = spD.tile([P, 512], F32, tag="dummy", name="opj2")
                nc.tensor.matmul(Pps2, lhs, wos[:, 512:1024],
                                 start=True, stop=True)
                nc.any.tensor_copy(ost[:, 512:1024], Pps2)
                nc.sync.dma_start(out[t0:t0 + P, :], ost)

            def normalize(hlo, q0, Ops, qlen):
                # den sits on PSUM partition 0 (ones col 0 of Vaug), so the
                # offset-blind fast reciprocal reads it directly; broadcast
                # on the PE (idle while it waits for this very chain) into
                # spP/spD; multiply O rows 64:128 straight out of PSUM
                rc = nrm.tile([1, 1024], F32, tag="rc", name="rc")[:, 0:qlen]
                nc.vector.reciprocal_approx_fast(rc, Ops[0:1, 0:qlen])
                rcb = nrm.tile([HD, 1024], F32, tag="rcb",
                               name="rcb")[:, 0:qlen]
                nc.gpsimd.partition_broadcast(rcb, rc)
                for j in range(qlen // 512):
                    nc.vector.tensor_tensor(
                        OtT[q0 // 512 + j][hlo:hlo + HD, :],
                        Ops[HD:P, ts(j, 512)],
                        rcb[:, ts(j, 512)],
                        mybir.AluOpType.mult)

            oproj_queue = []
            blocks = [(b, 1024 * nqb, 1024, h)
                      for b in (0, 1) for nqb in (0, 1) for h in (0, 1)]
            # split the final block into 512-q halves: the first half's
            # normalize releases 4 output-projection tiles one sub-block
            # earlier, halving the drain tail
            blocks = blocks[:-1] + [(1, 1024, 512, 1), (1, 1536, 512, 1)]
            pend = []        # (pv_fn, chunk_idx, Pt) pipeline carry-over
            prev_ctx = None  # (hlo, q0, Ops, qlen, bi) awaiting normalize
            for bi, (b, lq0, qlen, h) in enumerate(blocks):
                q0 = b * NB + lq0
                Vaug = VaugA[b] if h == 0 else VaugB[b]
                Kz = KzA[b] if h == 0 else KzB[b]
                Qb = Qt[b][lq0 // 1024]
                hlo = h * HD

                def s_exp(i, Kz=Kz, Qb=Qb, lq0=lq0, qlen=qlen):
                    Sps = spS.tile([P, 1024], F32, tag="S", name="Sps")
                    for hf in range(qlen // 512):
                        nc.tensor.matmul(
                            Sps[:, ts(hf, 512)],
                            Kz[i // 8][:, ts(i % 8, P)],
                            Qb[:, ds(lq0 % 1024 + hf * 512, 512)],
                            start=True, stop=True)
                    Pt = ptp.tile([P, 1024], BF16, tag="P", name="Pt")
                    nc.scalar.activation(
                        Pt[:, 0:qlen], Sps[:, 0:qlen],
                        mybir.ActivationFunctionType.Exp)
                    return Pt

                # the pipeline carries ACROSS block boundaries: issue this
                # block's first DEPTH S/exp chunks interleaved with the
                # previous block's tail PVs, then its normalize, so neither
                # the PE nor ScalarE drains between blocks
                DEPTH = 5
                sw = side_work.get(bi, [])
                cool = 0
                first_pts = []
                for k in range(DEPTH):
                    first_pts.append(s_exp(k))
                    if pend:
                        f, idx, pt = pend.pop(0)
                        f(idx, pt)
                    if cool > 0:
                        cool -= 1
                    elif sw:
                        fn, cool = sw.pop(0)
                        fn()
                    elif oproj_queue and bi - oproj_queue[0][1] >= 2:
                        oproj_tile(oproj_queue.pop(0)[0])
                if prev_ctx is not None:
                    phlo, pq0, pOps, pqlen, pbi = prev_ctx
                    normalize(phlo, pq0, pOps, pqlen)
                    if phlo:     # both heads of this q-range now normalized
                        for tch in range(pqlen // P):
                            oproj_queue.append((pq0 + tch * P, pbi))

                Ops = spO.tile([P, 1024], F32, tag="O", name="Ops")

                def pv(i, Pt, Vaug=Vaug, Ops=Ops, qlen=qlen):
                    for hf in range(qlen // 512):
                        nc.tensor.matmul(
                            Ops[:, ts(hf, 512)],
                            Vaug[i // 8][:, i % 8, :],
                            Pt[:, ts(hf, 512)],
                            start=(i == 0), stop=(i == 15),
                            skip_group_check=True)

                pend = [(pv, k, first_pts[k]) for k in range(DEPTH)]
                for i in range(DEPTH, 16):
                    pend.append((pv, i, s_exp(i)))
                    f, idx, pt = pend.pop(0)
                    f(idx, pt)
                    min_age = 2 if bi < len(blocks) - 1 else 1
                    if cool > 0:
                        cool -= 1
                    elif sw:
                        fn, cool = sw.pop(0)
                        fn()
                    elif oproj_queue and bi - oproj_queue[0][1] >= min_age:
                        oproj_tile(oproj_queue.pop(0)[0])
                while sw:
                    fn, cool = sw.pop(0)
                    fn()
                prev_ctx = (hlo, q0, Ops, qlen, bi)

            # drain the last block's pipeline + normalize
            for f, idx, pt in pend:
                f(idx, pt)
            phlo, pq0, pOps, pqlen, pbi = prev_ctx
            normalize(phlo, pq0, pOps, pqlen)
            for tch in range(pqlen // P):
                oproj_queue.append((pq0 + tch * P, pbi))
            # remaining output projections round-robin over four PSUM
            # banks so four matmul->copy chains overlap
            dr2 = spS.tile([P, 1024], F32, tag="S", name="dr2")
            dr3 = spS.tile([P, 1024], F32, tag="S", name="dr3")
            drO = spO.tile([P, 1024], F32, tag="O", name="drO")
            drain_banks = [dr2, dr3, drO]
            for dbi, (t0, _) in enumerate(oproj_queue):
                lhs = OtT[t0 // 512][:, ts((t0 % 512) // P, P)]
                ost = osb.tile([P, HID], BF16, tag="ost", name="ost")
                bank = drain_banks[dbi % 3]
                for hf in range(2):
                    nc.tensor.matmul(bank[:, ts(hf, 512)], lhs,
                                     wos[:, ts(hf, 512)],
                                     start=True, stop=True,
                                     skip_group_check=True)
                if dbi % 2 == 0:
                    nc.scalar.activation(
                        ost, bank, mybir.ActivationFunctionType.Identity)
                else:
                    nc.vector.tensor_copy(ost, bank)
                nc.sync.dma_start(out[t0:t0 + P, :], ost)

    nc.compile()
    return nc


def _get_nc():
    global _NC_CACHE
    if _NC_CACHE is None:
        _NC_CACHE = build_nc()
    return _NC_CACHE


def shard_inputs(x, rope_cos, rope_sin, Wq, bq, Wk, bk, Wv, bv, Wo, bo):
    """Build per-core input maps."""
    # [p, chunk, o, n_tail]: per partition, one contiguous 16KB chunk line
    xt = np.ascontiguousarray(
        x.reshape(NT, HID).T.reshape(8, P, 8, 512).transpose(1, 2, 0, 3)
    ).astype(np.float32)
    cosT = np.ascontiguousarray(rope_cos.reshape(NT, HD).T).astype(np.float32)
    sinT = np.ascontiguousarray(rope_sin.reshape(NT, HD).T).astype(np.float32)
    cos_id = np.ones((HD, NT), np.float32)
    sin_id = np.zeros((HD, NT), np.float32)
    # rotate_half as matrix R: out = R @ t, R[2i,2i+1]=-1, R[2i+1,2i]=+1.
    # matmul computes lhsT.T @ rhs, so pass R.T.
    R = np.zeros((P, P), np.float32)
    idx = np.arange(0, HD, 2)
    R[idx, idx + 1] = -1.0
    R[idx + 1, idx] = 1.0
    rmat = np.ascontiguousarray(R.T)

    in_maps = []
    for c in range(N_CORES):
        lo, hi = c * P, (c + 1) * P
        in_maps.append({
            "xt": xt,
            "wq": np.ascontiguousarray(
                Wq[:, lo:hi].reshape(8, P, P).transpose(1, 0, 2)
            ).astype(np.float32),
            "wk": np.ascontiguousarray(
                Wk[:, lo:hi].reshape(8, P, P).transpose(1, 0, 2)
            ).astype(np.float32),
            "wv": np.ascontiguousarray(
                Wv[:, lo:hi].reshape(8, P, P).transpose(1, 0, 2)
            ).astype(np.float32),
            "wo": np.ascontiguousarray(Wo[lo:hi, :]).astype(ml_dtypes.bfloat16),
            "bias": np.ascontiguousarray(
                np.stack([bq[lo:hi], bk[lo:hi], bv[lo:hi]], axis=1)
            ).astype(np.float32),
            "cos": (cosT if c == 0 else cos_id).astype(ml_dtypes.bfloat16),
            "sin": (sinT if c == 0 else sin_id).astype(ml_dtypes.bfloat16),
            "rmat": rmat,
        })
    return in_maps


def run_device(inputs, trace=False, **kw):
    nc = _get_nc()
    in_maps = shard_inputs(**inputs)
    res = run_bass_kernel_spmd(nc, in_maps, core_ids=list(range(N_CORES)),
                               trace=trace, **kw)
    return res


def gather(res, bo):
    acc = res.results[0]["out"].astype(np.float32)
    for c in range(1, N_CORES):
        acc = acc + res.results[c]["out"].astype(np.float32)
    acc += bo[None, :].astype(np.float32)
    return acc.reshape(2, NB, HID)


def kernel(**inputs):
    # NRT_EXEC_UNIT_UNRECOVERABLE crashes are transient on this fleet;
    # one retry rescues the run.
    try:
        res = run_device(inputs, trace=False)
    except Exception:
        res = run_device(inputs, trace=False)
    return gather(res, np.asarray(inputs["bo"], np.float32))


# revision 42
# speedup vs baseline: 1.1652x; 1.1652x over previous
"""Distributed Trainium2 kernel for nn_Attention_14697378086932.

Head-sharded (tensor-parallel) multi-head attention over 8 NeuronCores:
each core computes 2 of the 16 heads end-to-end.

Phase A (serial prefix, jointly DMA/PE-paced at ~420GB/s): QKV
projections for both batches, 512-token chunk pairs.  x^T arrives
host-rearranged so each chunk is one contiguous 16KB line per partition
(fewest DMA descriptors), split into two o-half tiles so the first
matmul waits on 1MB, not 2MB; 7 half-tile buffers keep the DMA engines
streaming continuously.  K's activation writes straight into the
zero-padded per-head KzA/KzB tiles, V is written bf16 and PE-transposed
(one bf16 transpose per 128-token chunk) into the per-key-chunk
[token, 1|pad|64ch] PV layout whose ones column drops the softmax
denominator onto PSUM partition 0.  rotate_half is a permutation
matmul; cores 1..7 get cos=1/sin=0 so rope degenerates to the
identity.  Pair p-1's rope runs BEFORE pair p's projections and its V
transposes after them, so every PSUM-bank WAR clears inside the
DMA-paced stream and the in-order PE queue never stalls.

Attention blocks (ScalarE exp-bound, ~16.4us per 1024-q block):
flash-style over 128-token key chunks; S^T = Kz Q^T (f32r), P^T =
exp(S^T) on ScalarE, O^T = [1|V]^T P^T (bf16) accumulating rows 64:128
of PSUM with den on row 0.  The S/exp/PV pipeline carries across block
boundaries (DEPTH=5); normalize = offset-blind fast approx reciprocal
read directly from PSUM row 0, gpsimd partition broadcast, and one
PSUM-direct multiply into the 512-token-granular OtT tiles.  Output
projections (O_loc @ Wo per 128-token chunk, stationary-shared halves
on the spP/spD banks, bf16 partials) inject into later blocks at age
>= 2; the final block is split into 512-q halves so only 4 tiles drain
at the end, round-robin over three PSUM banks with copies alternating
ScalarE/DVE.  Host sums the 8 partial outputs and adds bo.

All tiles are split per batch (and Kz/Vaug/Qt per 1024-token half) so
Tile's per-tile dependency tracking never chains a consumer onto an
unrelated producer.  All matmuls keep a full 128-row/column PE
footprint (the clock gate throttles half-array work to 1.2 GHz).
NOTE: the device alternates between two clock states (~1.2x apart), so
wall-clock exec times vary run to run; ~254-261us in the fast state.
"""
import sys

sys.path.insert(0, "/opt/trn_rl_repo")

import numpy as np
import ml_dtypes

import concourse.bass as bass
import concourse.mybir as mybir
from concourse import bacc
from concourse.bass import ts, ds
from concourse.tile import TileContext
from concourse.masks import make_identity
from concourse.bass_utils import run_bass_kernel_spmd

F32 = mybir.dt.float32
F32R = mybir.dt.float32r
BF16 = mybir.dt.bfloat16

P = 128          # partitions / local channels per core
HID = 1024       # hidden
NT = 4096        # total tokens (batch 2 x 2048)
NB = 2048        # tokens per batch
HD = 64          # head dim
N_CORES = 8

_NC_CACHE = None


def build_nc():
    nc = bacc.Bacc("TRN2")

    # xt: host-rearranged [p, chunk, o, n_tail]: each 512-token chunk is
    # one contiguous 16KB line per partition (fewest DMA descriptors)
    xt = nc.declare_dram_parameter("xt", [P, 8, 8, 512], F32R, isOutput=False)
    wq = nc.declare_dram_parameter("wq", [P, 8, P], F32R, isOutput=False)
    wk = nc.declare_dram_parameter("wk", [P, 8, P], F32R, isOutput=False)
    wv = nc.declare_dram_parameter("wv", [P, 8, P], F32R, isOutput=False)
    wo = nc.declare_dram_parameter("wo", [P, HID], BF16, isOutput=False)
    bia = nc.declare_dram_parameter("bias", [P, 3], F32, isOutput=False)
    cos = nc.declare_dram_parameter("cos", [HD, NT], BF16, isOutput=False)
    sin = nc.declare_dram_parameter("sin", [HD, NT], BF16, isOutput=False)
    rmat = nc.declare_dram_parameter("rmat", [P, P], F32R, isOutput=False)
    out = nc.declare_dram_parameter("out", [NT, HID], BF16, isOutput=True)

    with TileContext(nc) as tc:
        with tc.tile_pool(name="consts", bufs=1) as consts, \
             tc.tile_pool(name="big", bufs=1) as big, \
             tc.tile_pool(name="xtp", bufs=7) as xtp, \
             tc.tile_pool(name="ropet", bufs=1) as ropet, \
             tc.tile_pool(name="ptp", bufs=7) as ptp, \
             tc.tile_pool(name="osb", bufs=2) as osb, \
             tc.tile_pool(name="nrm", bufs=1) as nrm, \
             tc.tile_pool(name="spS", bufs=2, space="PSUM") as spS, \
             tc.tile_pool(name="spO", bufs=1, space="PSUM") as spO, \
             tc.tile_pool(name="spP", bufs=1, space="PSUM") as spP, \
             tc.tile_pool(name="spD", bufs=1, space="PSUM") as spD:

            # ---------------- constants (Activation HWDGE queue, parallel
            # with the xt stream on the SP queue)
            wqs = consts.tile([P, 8, P], F32R)
            wks = consts.tile([P, 8, P], F32R)
            wvs = consts.tile([P, 8, P], F32R)
            nc.scalar.dma_start(wqs, wq[:])
            nc.scalar.dma_start(wks, wk[:])
            nc.scalar.dma_start(wvs, wv[:])
            bias_t = consts.tile([P, 3], F32)
            nc.scalar.dma_start(bias_t, bia[:])
            rmat_t = consts.tile([P, P], F32R)
            nc.scalar.dma_start(rmat_t, rmat[:])
            trig_t = consts.tile([P, NT], BF16)
            cos_t = trig_t[0:HD]
            sin_t = trig_t[HD:P]
            nc.scalar.dma_start(cos_t, cos[:])
            nc.scalar.dma_start(sin_t, sin[:])
            wos = consts.tile([P, HID], BF16)
            nc.scalar.dma_start(wos, wo[:])
            ident = consts.tile([P, P], BF16)
            make_identity(nc, ident)

            # ---------------- per-batch state, split into 1024-token
            # half tiles so consumers wait only on the half they read
            # Q^T per (batch, nq-half) [128 ch, 1024 tok]
            Qt = [[big.tile([P, 1024], F32R, name=f"Qt{b}{n}")
                   for n in range(2)] for b in range(2)]
            # zero-padded per-head K^T: head A in rows 0..63, head B in
            # rows 64..127; the K activation writes its halves directly
            # into these (no intermediate Kt tile)
            KzA = [[big.tile([P, 1024], F32R, name=f"KzA{b}{n}")
                    for n in range(2)] for b in range(2)]
            KzB = [[big.tile([P, 1024], F32R, name=f"KzB{b}{n}")
                    for n in range(2)] for b in range(2)]
            for b in range(2):
                for n in range(2):
                    nc.vector.memset(KzA[b][n][HD:P, :].bitcast(F32), 0.0)
                    nc.vector.memset(KzB[b][n][0:HD, :].bitcast(F32), 0.0)
            # V^T staging in bf16, shared across batches (batch 1's
            # projection rewrites it only after batch 0's transposes read it)
            _vtb = big.tile([P, NB], BF16, name="Vtb")
            Vtb = [_vtb, _vtb]
            # V in [token, chunk, 64 V | 1 | 63 zeros] layout per
            # (head, batch, 8-chunk half)
            VaugA = [[big.tile([P, 8, P], BF16, name=f"VaugA{b}{n}")
                      for n in range(2)] for b in range(2)]
            VaugB = [[big.tile([P, 8, P], BF16, name=f"VaugB{b}{n}")
                      for n in range(2)] for b in range(2)]
            # col 0 = ones (so the softmax denominator lands on PSUM
            # partition 0, readable by the offset-blind custom reciprocal
            # with no staging copy); V channels in cols 64:128
            for b in range(2):
                for V in (*VaugA[b], *VaugB[b]):
                    nc.vector.memset(V, 0.0)
                    nc.vector.memset(V[:, :, 0:1], 1.0)
            # normalized attention out^T, one tile per 512 tokens so
            # output projections unblock at fine grain
            OtT = [big.tile([P, 512], BF16, name=f"Ot{k}") for k in range(8)]

            # ---------------- building blocks
            def load_chunk(c):
                # one 512-token chunk of x^T as two [128, 4 o-chunks, 512]
                # tiles so the first projection matmuls wait on 1MB, not 2MB
                ta = xtp.tile([P, 4, 512], F32R, tag="xt", name="xta")
                nc.sync.dma_start(ta, xt[:, c, 0:4])
                tb = xtp.tile([P, 4, 512], F32R, tag="xt", name="xtb")
                nc.sync.dma_start(tb, xt[:, c, 4:8])
                return (ta, tb)

            def qkv_pair(xtt0, xtt1, wt, acc, c0):
                # one projection for a pair of 512-token chunks into a
                # 2-bank accumulator; paired matmuls share their stationary
                # so every second one skips LDWEIGHTS
                for o in range(8):
                    nc.tensor.matmul(acc[:, 0:512], wt[:, o],
                                     xtt0[o // 4][:, o % 4],
                                     start=(o == 0), stop=(o == 7))
                    nc.tensor.matmul(acc[:, 512:1024], wt[:, o],
                                     xtt1[o // 4][:, o % 4],
                                     start=(o == 0), stop=(o == 7))

            def act_q(acc, b, l0, ln, dve=False):
                dst = Qt[b][l0 // 1024][:, ds(l0 % 1024, ln)]
                if dve:
                    nc.vector.tensor_scalar_add(dst, acc, bias_t[:, 0:1])
                else:
                    nc.scalar.activation(dst, acc,
                                         mybir.ActivationFunctionType.Identity,
                                         bias=bias_t[:, 0:1])

            def act_k(acc, b, l0, ln, dve=False):
                dA = KzA[b][l0 // 1024][0:HD, ds(l0 % 1024, ln)]
                dB = KzB[b][l0 // 1024][HD:P, ds(l0 % 1024, ln)]
                if dve:
                    nc.vector.tensor_scalar_add(dA, acc[0:HD],
                                                bias_t[0:HD, 1:2])
                    nc.vector.tensor_scalar_add(dB, acc[HD:P],
                                                bias_t[HD:P, 1:2])
                else:
                    nc.scalar.activation(dA, acc[0:HD],
                                         mybir.ActivationFunctionType.Identity,
                                         bias=bias_t[0:HD, 1:2])
                    nc.scalar.activation(dB, acc[HD:P],
                                         mybir.ActivationFunctionType.Identity,
                                         bias=bias_t[HD:P, 1:2])

            def act_v(acc, b, l0, ln, dve=False):
                if dve:
                    nc.vector.tensor_scalar_add(Vtb[b][:, ds(l0, ln)], acc,
                                                bias_t[:, 2:3])
                else:
                    nc.scalar.activation(Vtb[b][:, ds(l0, ln)], acc,
                                         mybir.ActivationFunctionType.Identity,
                                         bias=bias_t[:, 2:3])

            def rope_slice(tiles, b, l0, psr):
                # rope rows 0..63 of the 512-token slice at local offset l0
                t = tiles[b][l0 // 1024]
                lsl = ds(l0 % 1024, 512)
                gsl = ds(b * NB + l0, 512)
                nc.tensor.matmul(psr, rmat_t, t[:, lsl],
                                 start=True, stop=True)
                tmp = ropet.tile([HD, 512], F32, tag="tmp", name="tmp")
                nc.vector.tensor_tensor(tmp, psr[0:HD], sin_t[:, gsl],
                                        mybir.AluOpType.mult)
                nc.vector.tensor_tensor(t[0:HD, lsl], t[0:HD, lsl],
                                        cos_t[:, gsl], mybir.AluOpType.mult)
                nc.vector.tensor_tensor(t[0:HD, lsl], t[0:HD, lsl], tmp,
                                        mybir.AluOpType.add)

            def vtrans_chunk(b, kc, pst_bf, slot):
                # V [ch, tok] -> [tok, ch] per 128-token chunk: one bf16 PE
                # transpose (both heads at once) into a bf16 view of a
                # shared PSUM bank, then two copies into the Vaug layouts
                dst = pst_bf[:, ts(slot, P)]
                nc.tensor.transpose(dst, Vtb[b][:, ts(kc, P)], ident)
                nc.vector.tensor_copy(VaugA[b][kc // 8][:, kc % 8, HD:P],
                                      dst[:, 0:HD])
                nc.vector.tensor_copy(VaugB[b][kc // 8][:, kc % 8, HD:P],
                                      dst[:, HD:P])

            # ---------------- phase A: projections for both batches
            # (serial prefix, DMA-bound).  Software pipeline: pair p-1's
            # rope runs BEFORE pair p's projection matmuls and its V
            # transposes after them, so every PSUM-bank WAR clears during
            # the long DMA-paced projection stream and the in-order PE
            # queue never stalls on the DVE chains.
            def rope_pair(pr):
                b, l0p = pr // 2, (pr % 2) * 1024
                for u in range(2):
                    l0 = l0p + u * 512
                    psr = spP.tile([P, 512], F32, tag="oproj", name="psrQ")
                    rope_slice(Qt, b, l0, psr)
                    psr2 = spD.tile([P, 512], F32, tag="dummy", name="psrK")
                    rope_slice(KzA, b, l0, psr2)

            def vtrans_pair(pr):
                b, kc0 = pr // 2, (pr % 2) * 8
                for u, (pool, tag) in enumerate(((spP, "oproj"),
                                                 (spD, "dummy"))):
                    pst = pool.tile([P, 512], F32, tag=tag, name="pstV")
                    pst_bf = pst.bitcast(BF16)
                    for s in range(4):
                        vtrans_chunk(b, kc0 + u * 4 + s, pst_bf, s)

            xtts = [load_chunk(c) for c in range(2)]
            for pr in range(4):           # chunk pairs per batch: 2 x 2
                b = pr // 2
                c0 = 2 * pr
                if pr > 0:
                    rope_pair(pr - 1)
                x0, x1 = xtts[0], xtts[1]
                accQ = spS.tile([P, 1024], F32, tag="S", name="accQ")
                qkv_pair(x0, x1, wqs, accQ, c0)
                accK = spS.tile([P, 1024], F32, tag="S", name="accK")
                qkv_pair(x0, x1, wks, accK, c0)
                accV = spO.tile([P, 1024], F32, tag="O", name="accV")
                qkv_pair(x0, x1, wvs, accV, c0)
                xtts = xtts[2:]
                for c in (c0 + 2, c0 + 3):
                    if c < 8:
                        xtts.append(load_chunk(c))
                l0p = (pr % 2) * 1024
                act_q(accQ, b, l0p, 1024)
                act_k(accK, b, l0p, 1024)
                act_v(accV, b, l0p, 1024)
                if pr > 0:
                    vtrans_pair(pr - 1)
            rope_pair(3)
            vtrans_pair(3)

            side_work = {}

            # ---------------- attention + output projection
            def oproj_tile(t0):
                # output projection of one 128-token chunk (both heads);
                # the two halves use different psum banks so the second
                # matmul never queues behind the first half's PSUM read
                lhs = OtT[t0 // 512][:, ts((t0 % 512) // P, P)]
                ost = osb.tile([P, HID], BF16, tag="ost", name="ost")
                Pps = spP.tile([P, 512], F32, tag="oproj", name="opj")
                nc.tensor.matmul(Pps, lhs, wos[:, 0:512],
                                 start=True, stop=True)
                nc.any.tensor_copy(ost[:, 0:512], Pps)
                Pps2 = spD.tile([P, 512], F32, tag="dummy", name="opj2")
                nc.tensor.matmul(Pps2, lhs, wos[:, 512:1024],
                                 start=True, stop=True)
                nc.any.tensor_copy(ost[:, 512:1024], Pps2)
                nc.sync.dma_start(out[t0:t0 + P, :], ost)

            def normalize(hlo, q0, Ops, qlen):
                # den sits on PSUM partition 0 (ones col 0 of Vaug), so the
                # offset-blind fast reciprocal reads it directly; broadcast
                # on the PE (idle while it waits for this very chain) into
                # spP/spD; multiply O rows 64:128 straight out of PSUM
                rc = nrm.tile([1, 1024], F32, tag="rc", name="rc")[:, 0:qlen]
                nc.vector.reciprocal_approx_fast(rc, Ops[0:1, 0:qlen])
                rcb = nrm.tile([HD, 1024], F32, tag="rcb",
                               name="rcb")[:, 0:qlen]
                nc.gpsimd.partition_broadcast(rcb, rc)
                for j in range(qlen // 512):
                    nc.vector.tensor_tensor(
                        OtT[q0 // 512 + j][hlo:hlo + HD, :],
                        Ops[HD:P, ts(j, 512)],
                        rcb[:, ts(j, 512)],
                        mybir.AluOpType.mult)

            oproj_queue = []
            blocks = [(b, 1024 * nqb, 1024, h)
                      for b in (0, 1) for nqb in (0, 1) for h in (0, 1)]
            # split the final block into 512-q halves: the first half's
            # normalize releases 4 output-projection tiles one sub-block
            # earlier, halving the drain tail
            blocks = blocks[:-1] + [(1, 1024, 512, 1), (1, 1536, 512, 1)]
            pend = []        # (pv_fn, chunk_idx, Pt) pipeline carry-over
            prev_ctx = None  # (hlo, q0, Ops, qlen, bi) awaiting normalize
            for bi, (b, lq0, qlen, h) in enumerate(blocks):
                q0 = b * NB + lq0
                Vaug = VaugA[b] if h == 0 else VaugB[b]
                Kz = KzA[b] if h == 0 else KzB[b]
                Qb = Qt[b][lq0 // 1024]
                hlo = h * HD

                def s_exp(i, Kz=Kz, Qb=Qb, lq0=lq0, qlen=qlen):
                    Sps = spS.tile([P, 1024], F32, tag="S", name="Sps")
                    for hf in range(qlen // 512):
                        nc.tensor.matmul(
                            Sps[:, ts(hf, 512)],
                            Kz[i // 8][:, ts(i % 8, P)],
                            Qb[:, ds(lq0 % 1024 + hf * 512, 512)],
                            start=True, stop=True)
                    Pt = ptp.tile([P, 1024], BF16, tag="P", name="Pt")
                    nc.scalar.activation(
                        Pt[:, 0:qlen], Sps[:, 0:qlen],
                        mybir.ActivationFunctionType.Exp)
                    return Pt

                # the pipeline carries ACROSS block boundaries: issue this
                # block's first DEPTH S/exp chunks interleaved with the
                # previous block's tail PVs, then its normalize, so neither
                # the PE nor ScalarE drains between blocks
                DEPTH = 5
                sw = side_work.get(bi, [])
                cool = 0
                first_pts = []
                for k in range(DEPTH):
                    first_pts.append(s_exp(k))
                    if pend:
                        f, idx, pt = pend.pop(0)
                        f(idx, pt)
                    if cool > 0:
                        cool -= 1
                    elif sw:
                        fn, cool = sw.pop(0)
                        fn()
                    elif oproj_queue and bi - oproj_queue[0][1] >= 2:
                        oproj_tile(oproj_queue.pop(0)[0])
                if prev_ctx is not None:
                    phlo, pq0, pOps, pqlen, pbi = prev_ctx
                    normalize(phlo, pq0, pOps, pqlen)
                    if phlo:     # both heads of this q-range now normalized
                        for tch in range(pqlen // P):
                            oproj_queue.append((pq0 + tch * P, pbi))

                Ops = spO.tile([P, 1024], F32, tag="O", name="Ops")

                def pv(i, Pt, Vaug=Vaug, Ops=Ops, qlen=qlen):
                    for hf in range(qlen // 512):
                        nc.tensor.matmul(
                            Ops[:, ts(hf, 512)],
                            Vaug[i // 8][:, i % 8, :],
                            Pt[:, ts(hf, 512)],
                            start=(i == 0), stop=(i == 15),
                            skip_group_check=True)

                pend = [(pv, k, first_pts[k]) for k in range(DEPTH)]
                for i in range(DEPTH, 16):
                    pend.append((pv, i, s_exp(i)))
                    f, idx, pt = pend.pop(0)
                    f(idx, pt)
                    min_age = 2 if bi < len(blocks) - 1 else 1
                    if cool > 0:
                        cool -= 1
                    elif sw:
                        fn, cool = sw.pop(0)
                        fn()
                    elif oproj_queue and bi - oproj_queue[0][1] >= min_age:
                        oproj_tile(oproj_queue.pop(0)[0])
                while sw:
                    fn, cool = sw.pop(0)
                    fn()
                prev_ctx = (hlo, q0, Ops, qlen, bi)

            # drain the last block's pipeline + normalize
            for f, idx, pt in pend:
                f(idx, pt)
            phlo, pq0, pOps, pqlen, pbi = prev_ctx
            normalize(phlo, pq0, pOps, pqlen)
            for tch in range(pqlen // P):
                oproj_queue.append((pq0 + tch * P, pbi))
            # remaining output projections round-robin over four PSUM
            # banks so four matmul->copy chains overlap
            dr2 = spS.tile([P, 1024], F32, tag="S", name="dr2")
            dr3 = spS.tile([P, 1024], F32, tag="S", name="dr3")
            drO = spO.tile([P, 1024], F32, tag="O", name="drO")
            drain_banks = [dr2, dr3, drO]
            for dbi, (t0, _) in enumerate(oproj_queue):
                lhs = OtT[t0 // 512][:, ts((t0 % 512) // P, P)]
                ost = osb.tile([P, HID], BF16, tag="ost", name="ost")
                bank = drain_banks[dbi % 3]
                for hf in range(2):
                    nc.tensor.matmul(bank[:, ts(hf, 512)], lhs,
                                     wos[:, ts(hf, 512)],
                                     start=True, stop=True,
                                     skip_group_check=True)
                if dbi % 2 == 0:
                    nc.scalar.activation(
                        ost, bank, mybir.ActivationFunctionType.Identity)
                else:
                    nc.vector.tensor_copy(ost, bank)
                nc.sync.dma_start(out[t0:t0 + P, :], ost)

    nc.compile()
    return nc


def _get_nc():
    global _NC_CACHE
    if _NC_CACHE is None:
        _NC_CACHE = build_nc()
    return _NC_CACHE


def shard_inputs(x, rope_cos, rope_sin, Wq, bq, Wk, bk, Wv, bv, Wo, bo):
    """Build per-core input maps."""
    # [p, chunk, o, n_tail]: per partition, one contiguous 16KB chunk line
    xt = np.ascontiguousarray(
        x.reshape(NT, HID).T.reshape(8, P, 8, 512).transpose(1, 2, 0, 3)
    ).astype(np.float32)
    cosT = np.ascontiguousarray(rope_cos.reshape(NT, HD).T).astype(np.float32)
    sinT = np.ascontiguousarray(rope_sin.reshape(NT, HD).T).astype(np.float32)
    cos_id = np.ones((HD, NT), np.float32)
    sin_id = np.zeros((HD, NT), np.float32)
    # rotate_half as matrix R: out = R @ t, R[2i,2i+1]=-1, R[2i+1,2i]=+1.
    # matmul computes lhsT.T @ rhs, so pass R.T.
    R = np.zeros((P, P), np.float32)
    idx = np.arange(0, HD, 2)
    R[idx, idx + 1] = -1.0
    R[idx + 1, idx] = 1.0
    rmat = np.ascontiguousarray(R.T)

    in_maps = []
    for c in range(N_CORES):
        lo, hi = c * P, (c + 1) * P
        in_maps.append({
            "xt": xt,
            "wq": np.ascontiguousarray(
                Wq[:, lo:hi].reshape(8, P, P).transpose(1, 0, 2)
            ).astype(np.float32),
            "wk": np.ascontiguousarray(
                Wk[:, lo:hi].reshape(8, P, P).transpose(1, 0, 2)
            ).astype(np.float32),
            "wv": np.ascontiguousarray(
                Wv[:, lo:hi].reshape(8, P, P).transpose(1, 0, 2)
            ).astype(np.float32),
            "wo": np.ascontiguousarray(Wo[lo:hi, :]).astype(ml_dtypes.bfloat16),
            "bias": np.ascontiguousarray(
                np.stack([bq[lo:hi], bk[lo:hi], bv[lo:hi]], axis=1)
            ).astype(np.float32),
            "cos": (cosT if c == 0 else cos_id).astype(ml_dtypes.bfloat16),
            "sin": (sinT if c == 0 else sin_id).astype(ml_dtypes.bfloat16),
            "rmat": rmat,
        })
    return in_maps


def run_device(inputs, trace=False, **kw):
    nc = _get_nc()
    in_maps = shard_inputs(**inputs)
    res = run_bass_kernel_spmd(nc, in_maps, core_ids=list(range(N_CORES)),
                               trace=trace, **kw)
    return res


def gather(res, bo):
    acc = res.results[0]["out"].astype(np.float32)
    for c in range(1, N_CORES):
        acc = acc + res.results[c]["out"].astype(np.float32)
    acc += bo[None, :].astype(np.float32)
    return acc.reshape(2, NB, HID)


def kernel(**inputs):
    # NRT_EXEC_UNIT_UNRECOVERABLE crashes are transient on this fleet;
    # one retry rescues the run.
    try:
        res = run_device(inputs, trace=False)
    except Exception:
        res = run_device(inputs, trace=False)
    return gather(res, np.asarray(inputs["bo"], np.float32))
